# revision 17
# baseline (speedup 1.0000x reference)
"""Trainium2 Bass kernel for a 3-layer LSTM (B=4096, T=1024, IN=2, H=16) + final FC.

Per core (batch-sharded 8 ways, B_local=512), wavefront over layers:
macro-step s computes L0@t=s, L1@t=s-1, L2@t=s-2. The batch is further split
into 2 staggered chunks of 256 so two independent recurrence chains can
interleave across engines (hides the per-step dependency-chain latency).

Unified stationary window: S rows 0:16 h0, 16:32 h1, 32:48 h2, 48:50 x_t,
50 one(bias) -> ONE K=51 moving operand feeds both matmuls; no h1
duplication needed, and the combined h-write lands at S[0:48] (BIR partition
rule: an AP must not straddle the 64-partition boundary unless it starts
at 0 or 64).

Gate layout (PSUM P [128, 2, 256] f32 = 2 banks):
  bank0 (MM_A): rows 0:48 = i (i0,i1,i2), rows 64:112 = f (f0,f1,f2)
  bank1 (MM_B): rows 0:48 = 2*g, rows 64:112 = o   (g prescaled x2 in weights)
Per chunk-step (all elementwise ops are single fused 48-row ops; TT/STT
SBUF inputs must share their base partition - layout built around that):
  MM_A -> sigA [128,256] ACT  (i,f)   } sigA only needs MM_A, so FW can
  MM_B -> sigB [128,256] ACT  (2g,o)  } start while MM_B/sigB are in flight
  FW  = G_f * C                 [48,256]  DVE tensor_mul (1x, f32 c)
  HU  = (G_g - 0.5) * G_i       [48,256]  DVE scalar_tensor_tensor
  C'  = 2*HU + FW               [48,256]  DVE scalar_tensor_tensor
  TC  = tanh(C')                [48,256]  ACT
  S[0:48] = G_o * TC  (h0,h1,h2 at once) [48,256]  DVE (f16 2x mode)
(tanh(g) = 2*sigmoid(2g)-1 folded into the HU/C' scalars, so one table set
serves every activation; i*tanh(g) is computed as half-product then doubled.)
"""

import os
import sys

sys.path.insert(0, "/opt/trn_rl_repo")

import numpy as np

import concourse.bacc as bacc
import concourse.mybir as mybir
from concourse.tile import TileContext
from concourse import bass_utils

B, T, IN, H, L = 4096, 1024, 2, 16, 3
NCORES = 8
BL = B // NCORES          # 512
NCHUNK = int(os.environ.get("LSTM_NCHUNK", "2"))
CB = BL // NCHUNK
F32 = mybir.dt.float32
F16 = mybir.dt.float16
NPF16 = np.float16

# PyTorch gate rows in W_ih*/W_hh*: i, f, g, o
PT_I, PT_F, PT_G, PT_O = slice(0, 16), slice(16, 32), slice(32, 48), slice(48, 64)

_STEPS_ENV = int(os.environ.get("LSTM_STEPS", "0"))
_FW_POOL = os.environ.get("LSTM_FW_POOL", "0") == "1"
_CDT_ENV = os.environ.get("LSTM_CDT", "f16")  # cell-state dtype: f32 | f16


def _t_run():
    return _STEPS_ENV if _STEPS_ENV > 0 else T


KW = 51  # stationary/moving contraction rows


def build_weight_block(W_ih0, W_hh0, b0, W_ih1, W_hh1, b1, W_ih2, W_hh2, b2):
    """WT [51, 256] f32. Cols 0:128 = MM_A (i,f), 128:256 = MM_B (2g, o).

    K rows: 0:16 h0, 16:32 h1, 32:48 h2, 48:50 x, 50 one(bias).
    Col layout within each matmul: layer-l gate block at 16*l : 16*l+16 for
    the first gate group (i or g), 64+16*l : 64+16*l+16 for the second (f or o).
    """
    WT = np.zeros((KW, 256), np.float32)
    layers = [
        # (in_rows, rec_rows, W_ih, W_hh, b)
        (slice(48, 50), slice(0, 16), W_ih0, W_hh0, b0),
        (slice(0, 16), slice(16, 32), W_ih1, W_hh1, b1),
        (slice(16, 32), slice(32, 48), W_ih2, W_hh2, b2),
    ]
    for half, (pt0, pt1) in ((0, (PT_I, PT_F)), (128, (PT_G, PT_O))):
        for l, (ir, rr, Wih, Whh, b) in enumerate(layers):
            for pt, cbase, sc in ((pt0, half + 16 * l, 2.0 if half else 1.0),
                                  (pt1, half + 64 + 16 * l, 1.0)):
                WT[ir, cbase:cbase + 16] = sc * Wih[pt].T
                WT[rr, cbase:cbase + 16] = sc * Whh[pt].T
                WT[50, cbase:cbase + 16] = sc * b[pt]
    return WT


def build_bass():
    CDT = F32 if _CDT_ENV == "f32" else F16
    NPC = np.float32 if _CDT_ENV == "f32" else NPF16
    nc = bacc.Bacc("TRN2", target_bir_lowering=False, debug=False,
                   num_devices=NCORES)
    NT = _t_run() + 2
    nblk = (_t_run() + 63) // 64

    wt_d = nc.dram_tensor("wt", [KW, 256], F16, kind="ExternalInput")
    wfc_d = nc.dram_tensor("wfc", [17, 1], F16, kind="ExternalInput")
    xt_d, s0_d, c0_d, h1i_d, c1i_d, h2i_d, c2i_d = [], [], [], [], [], [], []
    for k in range(NCHUNK):
        xt_d.append(nc.dram_tensor(f"xt{k}", [128, nblk * CB], F16,
                                   kind="ExternalInput"))
        s0_d.append(nc.dram_tensor(f"s0{k}", [KW, CB], F16,
                                   kind="ExternalInput"))
        c0_d.append(nc.dram_tensor(f"c0{k}", [48, CB],
                                   mybir.dt.from_np(np.dtype(NPC)),
                                   kind="ExternalInput"))
        h1i_d.append(nc.dram_tensor(f"h1i{k}", [16, CB], F16,
                                    kind="ExternalInput"))
        c1i_d.append(nc.dram_tensor(f"c1i{k}", [16, CB],
                                    mybir.dt.from_np(np.dtype(NPC)),
                                    kind="ExternalInput"))
        h2i_d.append(nc.dram_tensor(f"h2i{k}", [16, CB], F16,
                                    kind="ExternalInput"))
        c2i_d.append(nc.dram_tensor(f"c2i{k}", [16, CB],
                                    mybir.dt.from_np(np.dtype(NPC)),
                                    kind="ExternalInput"))
    y_d = nc.dram_tensor("y", [1, BL], F32, kind="ExternalOutput")

    SIG = mybir.ActivationFunctionType.Sigmoid
    TANH = mybir.ActivationFunctionType.Tanh
    ADD = mybir.AluOpType.add
    MULT = mybir.AluOpType.mult

    with TileContext(nc) as tc:
        wt = nc.alloc_sbuf_tensor("wt_sb", [KW, 256], F16)
        wfc = nc.alloc_sbuf_tensor("wfc_sb", [17, 1], F16)
        hf = nc.alloc_sbuf_tensor("hf_sb", [17, BL], F16)
        ys = nc.alloc_sbuf_tensor("ys_sb", [1, BL], F32)
        xt, S, C = [], [], []
        for k in range(NCHUNK):
            xt.append(nc.alloc_sbuf_tensor(f"xt_sb{k}", [128, nblk * CB], F16))
            S.append(nc.alloc_sbuf_tensor(f"S_sb{k}", [KW, CB], F16))
            C.append(nc.alloc_sbuf_tensor(f"C_sb{k}", [112, CB], CDT))

        nc.sync.dma_start(wt[:, :], wt_d.ap())
        nc.sync.dma_start(wfc[:, :], wfc_d.ap())
        for k in range(NCHUNK):
            nc.sync.dma_start(xt[k][:, :], xt_d[k].ap())
            nc.sync.dma_start(S[k][:, :], s0_d[k].ap())
            nc.sync.dma_start(C[k][64:112, :], c0_d[k].ap())

        psum_bufs = 2 if NCHUNK == 2 else 1
        with tc.tile_pool(name="ps", bufs=psum_bufs, space="PSUM") as pps, \
             tc.tile_pool(name="sb", bufs=4) as psb:
            # Emission order = scheduler priority (tie-break among ready
            # ops). Interleave the two chunks at op granularity so the DVE
            # queue order is HU(a),FW(a),C'(a),HU(b),FW(b),h(a),C'(b),h(b):
            # chunk b's front fills the C'(a)->tanh(a) latency gap, and h(a)
            # (which unblocks MM(a,s+1)) runs as soon as tanh(a) lands.
            fw_eng = nc.gpsimd if _FW_POOL else nc.vector
            live = [None] * NCHUNK  # per-chunk (G, HU, FW)

            def front(k):
                P = pps.tile([128, 2, 256], F32, tag=f"P{k}")
                G = psb.tile([128, 2 * CB], F16, tag=f"G{k}")
                HU = psb.tile([48, CB], F16, tag=f"HU{k}")
                FW = psb.tile([48, CB], CDT, tag=f"FW{k}")
                # Split sigmoid per bank: sigA (i,f) only needs MM_A, so FW
                # can start while MM_B/sigB (2g, o) are still in flight.
                # Both matmuls emitted adjacently so the PE queue keeps a
                # chunk's pair back-to-back (no cross-chunk wedge).
                nc.tensor.matmul(P[0:128, 0, 0:CB], wt[0:KW, 0:128],
                                 S[k][0:KW, :], start=True, stop=True)
                nc.tensor.matmul(P[0:128, 1, 0:CB], wt[0:KW, 128:256],
                                 S[k][0:KW, :], start=True, stop=True)
                nc.scalar.activation(G[0:128, 0:CB], P[0:128, 0, 0:CB], SIG)
                nc.scalar.activation(G[0:128, CB:2 * CB], P[0:128, 1, 0:CB],
                                     SIG)
                live[k] = (G, HU, FW)

            def mid(k):
                G, HU, FW = live[k]
                # fw = f * c   (needs sigA only)
                fw_eng.tensor_mul(FW[0:48, :], G[64:112, 0:CB],
                                  C[k][64:112, :])
                # hu = (sig(2g) - 0.5) * i   [= i*tanh(g)/2]
                nc.vector.scalar_tensor_tensor(
                    HU[0:48, :], G[0:48, CB:2 * CB], -0.5, G[0:48, 0:CB],
                    ADD, MULT)

            def cprime(k):
                G, HU, FW = live[k]
                TC = psb.tile([112, CB], F16, tag=f"TC{k}")
                # c' = 2*hu + fw
                nc.vector.scalar_tensor_tensor(
                    C[k][64:112, :], HU[0:48, :], 2.0, FW[0:48, :],
                    MULT, ADD)
                # tc = tanh(c')
                nc.scalar.activation(TC[64:112, :], C[k][64:112, :], TANH)
                live[k] = (G, TC)

            def hout(k, s):
                G, TC = live[k]
                # h0,h1,h2 = o * tc in one op
                nc.vector.tensor_mul(S[k][0:48, :], G[64:112, CB:2 * CB],
                                     TC[64:112, :])
                # stage next x
                if s + 1 < _t_run():
                    nb_, nu = divmod(s + 1, 64)
                    nc.sync.dma_start(
                        S[k][48:50, :],
                        xt[k][2 * nu:2 * nu + 2, nb_ * CB:(nb_ + 1) * CB])
                # delayed init: overwrite wavefront-startup pollution
                if s == 0:
                    nc.sync.dma_start(S[k][16:32, :], h1i_d[k].ap())
                    nc.sync.dma_start(C[k][80:96, :], c1i_d[k].ap())
                elif s == 1:
                    nc.sync.dma_start(S[k][32:48, :], h2i_d[k].ap())
                    nc.sync.dma_start(C[k][96:112, :], c2i_d[k].ap())

            for m in range(NT):
                if NCHUNK == 2:
                    front(0)
                    front(1)
                    mid(0)
                    cprime(0)
                    hout(0, m)
                    mid(1)
                    cprime(1)
                    hout(1, m)
                else:
                    for k in range(NCHUNK):
                        front(k)
                    for k in range(NCHUNK):
                        mid(k)
                    cprime(0)
                    for k in range(1, NCHUNK):
                        cprime(k)
                        hout(k - 1, m)
                    hout(NCHUNK - 1, m)

        # final fc: y = h2 @ W_fc.T + b_fc
        with tc.tile_pool(name="pf", bufs=1, space="PSUM") as ppf:
            nc.vector.memset(hf[0:17, :], 1.0)
            for k in range(NCHUNK):
                nc.vector.tensor_copy(hf[0:16, k * CB:(k + 1) * CB],
                                      S[k][32:48, :])
            PF = ppf.tile([1, BL], F32, tag="PF")
            nc.tensor.matmul(PF[0:1, :], wfc[0:17, 0:1], hf[0:17, :],
                             start=True, stop=True)
            nc.scalar.copy(ys[0:1, :], PF[0:1, :])
            nc.sync.dma_start(y_d.ap(), ys[0:1, :])

    nc.compile()
    return nc


def prep_chunk_inputs(inputs, core, k):
    NPC = np.float32 if _CDT_ENV == "f32" else NPF16
    b0 = core * BL + k * CB
    b1 = b0 + CB
    tr = _t_run()
    nblk = (tr + 63) // 64

    x = np.asarray(inputs["x"])[b0:b1]          # [CB, T, IN]
    h0 = np.asarray(inputs["h0"])[:, b0:b1]     # [L, CB, H]
    c0 = np.asarray(inputs["c0"])[:, b0:b1]

    # xt layout: partition = 2*(t%64)+f, free = (t//64)*CB + b
    xt = np.zeros((128, nblk * CB), np.float32)
    xr = x[:, :tr, :].transpose(1, 2, 0)         # [t, f, b]
    for tb in range(nblk):
        t1 = min(tb * 64 + 64, tr)
        chunk = xr[tb * 64:t1]                   # [u, f, b]
        xt[:2 * (t1 - tb * 64), tb * CB:(tb + 1) * CB] = chunk.reshape(-1, CB)

    s0 = np.zeros((KW, CB), np.float32)
    s0[0:16] = h0[0].T
    s0[16:32] = h0[1].T
    s0[32:48] = h0[2].T
    s0[48:50] = x[:, 0, :].T
    s0[50] = 1.0

    c0p = np.concatenate([c0[0].T, c0[1].T, c0[2].T], axis=0)  # [48, CB]

    return {
        f"xt{k}": xt.astype(NPF16),
        f"s0{k}": s0.astype(NPF16),
        f"c0{k}": np.ascontiguousarray(c0p).astype(NPC),
        f"h1i{k}": np.ascontiguousarray(h0[1].T).astype(NPF16),
        f"c1i{k}": np.ascontiguousarray(c0[1].T).astype(NPC),
        f"h2i{k}": np.ascontiguousarray(h0[2].T).astype(NPF16),
        f"c2i{k}": np.ascontiguousarray(c0[2].T).astype(NPC),
    }


_NC_CACHE = {}


def kernel(**inputs):
    key = (_t_run(), _CDT_ENV, _FW_POOL, NCHUNK)
    if key not in _NC_CACHE:
        _NC_CACHE[key] = build_bass()
    nc = _NC_CACHE[key]

    b0v = np.asarray(inputs["b_ih0"]) + np.asarray(inputs["b_hh0"])
    b1v = np.asarray(inputs["b_ih1"]) + np.asarray(inputs["b_hh1"])
    b2v = np.asarray(inputs["b_ih2"]) + np.asarray(inputs["b_hh2"])
    WT = build_weight_block(
        np.asarray(inputs["W_ih0"]), np.asarray(inputs["W_hh0"]), b0v,
        np.asarray(inputs["W_ih1"]), np.asarray(inputs["W_hh1"]), b1v,
        np.asarray(inputs["W_ih2"]), np.asarray(inputs["W_hh2"]), b2v,
    ).astype(NPF16)
    wfc = np.zeros((17, 1), np.float32)
    wfc[0:16, 0] = np.asarray(inputs["W_fc"])[0]
    wfc[16, 0] = np.asarray(inputs["b_fc"])[0]
    wfc = wfc.astype(NPF16)

    in_maps = []
    for core in range(NCORES):
        m = {"wt": WT, "wfc": wfc}
        for k in range(NCHUNK):
            m.update(prep_chunk_inputs(inputs, core, k))
        in_maps.append(m)

    trace = os.environ.get("LSTM_TRACE", "0") == "1"
    res = bass_utils.run_bass_kernel_spmd(nc, in_maps, core_ids=list(range(NCORES)),
                                          trace=trace)
    global _LAST_RESULT
    _LAST_RESULT = res
    out = np.concatenate([res.results[c]["y"][0] for c in range(NCORES)])
    return out.reshape(B, 1).astype(np.float32)


_LAST_RESULT = None


if __name__ == "__main__":
    import reference
    inputs = reference.setup_inputs()
    y = kernel(**{k: np.asarray(v) for k, v in inputs.items()})
    print("kernel out", y.shape, y[:4, 0])
